# revision 24
# baseline (speedup 1.0000x reference)
"""Grouped-expert FFN (MoE) Trainium2 kernel.

Problem: E=64 experts, each x[1024,512] @ w1[512,2048] -> +b1 -> gelu(erf)
-> @ w2[2048,512] -> +b2, rows >= valid_load[e] zeroed.

Strategy:
 - Expert parallelism over 8 cores, flat "position" schedule: each expert's
   valid rows (8-rounded) are cut into <=512-column pieces; pieces are
   sorted by width and dealt 8-at-a-time into positions. All cores run the
   same program (position widths = max of its 8 pieces), each position
   loads its own expert weights, so load balance is near-optimal
   (4736 cols/core vs 4637 ideal). Cut points are tuned by a small
   deterministic local search to minimize the group-max sum.
 - Host transposes x per piece (xT [D,W]) so the device contracts over D
   with zero on-chip transposes: GEMM1 computes hT = w1.T-tiles @ xT
   (stationary w1 k/m tile, moving xT), GEMM2 computes yT = w2-tiles @ hT.
   Both biases land on the partition axis -> free via ACT activation bias.
 - bf16 matmul operands (PE streams bf16 at 1 elem/cell/cycle, same as
   fp32r, but weight DMA halves and fast-weight-load kicks in). PSUM
   accumulation stays fp32; biases fp32; y stored bf16, upcast on host.
 - Emission alternates wide/narrow positions so weight prefetch for cheap
   positions hides under the long computes; w1/x are loaded in k-slices
   (coarse, partition-contiguous) so the first matmuls start early without
   fine-grained DMA fighting the PE for SBUF; biases ride the ACT ring;
   the cold-start position's w1 is split across both HWDGE rings.
"""

import numpy as np
import ml_dtypes
import random

import concourse.bass as bass
import concourse.bacc as bacc
import concourse.tile as tile
from concourse import mybir
from concourse.bass_utils import run_bass_kernel_spmd

E, CAP, D, H = 64, 1024, 512, 2048
N_CORES = 8
CHUNK = 512                      # max columns of C per position
KTILES1 = D // 128               # 4  (contraction tiles of GEMM1)
MTILES1 = H // 128               # 16 (output partition tiles of GEMM1)
KTILES2 = H // 128               # 16 (contraction tiles of GEMM2)
MTILES2 = D // 128               # 4  (output partition tiles of GEMM2)

F32 = mybir.dt.float32
BF16 = mybir.dt.bfloat16
NP_BF16 = ml_dtypes.bfloat16

_PROGRAM_CACHE: dict[tuple, object] = {}
_SCHED_CACHE: dict[tuple, tuple] = {}
LAST_RESULT = None               # test harness introspection


def _r32(x):
    return -(-int(x) // 8) * 8


def _schedule(v):
    """Cut experts into <=512-wide pieces and group into positions.

    Returns (widths, emit_order, groups) where groups[g] is a list of up to
    8 pieces (expert, col_start, width) assigned to cores 0..len-1.
    """
    key = tuple(int(x) for x in v)
    if key in _SCHED_CACHE:
        return _SCHED_CACHE[key]
    v32 = [_r32(x) for x in v]
    big = [i for i in range(E) if v32[i] > CHUNK]
    FLOOR = 60  # below ~60 cols a position is LDW/dispatch-bound anyway

    def build(cuts):
        pieces = []
        for i in range(E):
            if v32[i] == 0:
                continue
            if v32[i] <= CHUNK:
                pieces.append((v32[i], i, 0))
            else:
                c = cuts[i]
                pieces.append((c, i, 0))
                pieces.append((v32[i] - c, i, c))
        return pieces

    def cost(cuts):
        w = sorted((p[0] for p in build(cuts)), reverse=True)
        P = -(-len(w) // 8)
        w = w + [0] * (P * 8 - len(w))
        return sum(max(max(w[g * 8:(g + 1) * 8]), FLOOR) for g in range(P))

    cuts = {i: CHUNK for i in big}
    rng = random.Random(7)
    cur, curc = dict(cuts), cost(cuts)
    best, bestc = dict(cur), curc
    steps = [-32, 32, -64, 64, -96, 96, -128, 128, -192, 192]
    for _ in range(30000 if big else 0):
        i = big[rng.randrange(len(big))]
        nv = cur[i] + steps[rng.randrange(len(steps))]
        if not (32 <= nv <= CHUNK and 32 <= v32[i] - nv <= CHUNK):
            continue
        nxt = dict(cur)
        nxt[i] = nv
        c2 = cost(nxt)
        if c2 <= curc:
            cur, curc = nxt, c2
            if c2 < bestc:
                best, bestc = dict(cur), c2

    pieces = sorted(build(best), reverse=True)
    P = -(-len(pieces) // 8)
    widths, groups = [], []
    for g in range(P):
        grp = pieces[g * 8:(g + 1) * 8]
        wmax = max(p[0] for p in grp)
        if wmax <= 0:
            continue
        widths.append(wmax)
        groups.append([(e, c0, w) for (w, e, c0) in grp])
    # emission order: alternate widest/narrowest so weight prefetch for
    # cheap positions hides under the long computes; first stays widest,
    # and the narrowest is saved for last so the final drain is short
    idx = list(range(len(widths)))
    order = []
    lo, hi = 0, len(idx) - 2
    while lo <= hi:
        order.append(idx[lo]); lo += 1
        if lo <= hi:
            order.append(idx[hi]); hi -= 1
    if len(idx) > 1:
        order.append(idx[-1])
    else:
        order = idx
    res = (tuple(widths), tuple(order), groups)
    _SCHED_CACHE[key] = res
    return res


def _build_program(widths: tuple, order: tuple):
    """One SPMD program; position p runs one chunk of widths[p] columns."""
    nc = bacc.Bacc(None, target_bir_lowering=False)
    P = len(widths)

    xt = nc.dram_tensor("xt", [P, D, CHUNK], BF16, kind="ExternalInput")
    w1g = nc.dram_tensor("w1g", [P, D, H], BF16, kind="ExternalInput")
    w2g = nc.dram_tensor("w2g", [P, H, D], BF16, kind="ExternalInput")
    bg = nc.dram_tensor("bg", [P, 128, MTILES1 + MTILES2], F32,
                        kind="ExternalInput")
    yt = nc.dram_tensor("yt", [P, D, CHUNK], BF16, kind="ExternalOutput")

    Gelu = mybir.ActivationFunctionType.Gelu
    Ident = mybir.ActivationFunctionType.Identity

    with tile.TileContext(nc) as tc:
        with (
            tc.tile_pool(name="w1p", bufs=3) as w1p,
            tc.tile_pool(name="w2p", bufs=3) as w2p,
            tc.tile_pool(name="bp", bufs=3) as bp,
            tc.tile_pool(name="xp", bufs=3) as xp,
            tc.tile_pool(name="hp", bufs=2) as hp,
            tc.tile_pool(name="yp", bufs=2) as yp,
            tc.tile_pool(name="ps_h", bufs=4, space="PSUM") as ps_h,
            tc.tile_pool(name="ps_y", bufs=4, space="PSUM") as ps_y,
        ):
            for pi, p in enumerate(order):
                W = widths[p]
                w1_s = w1g[p].rearrange("(k p) h -> p k h", p=128)
                w2_s = w2g[p].rearrange("(k p) d -> p k d", p=128)
                xt_s = xt[p].rearrange("(k p) c -> p k c", p=128)
                yt_s = yt[p].rearrange("(m p) c -> p m c", p=128)

                w1_t = w1p.tile([128, KTILES1, H], BF16, tag="w1")
                x_t = xp.tile([128, KTILES1, CHUNK], BF16, tag="x")
                # biases ride the second (ACT) HWDGE ring so they arrive
                # before the first gelu without delaying the weight stream
                b_t = bp.tile([128, MTILES1 + MTILES2], F32, tag="b")
                nc.scalar.dma_start(out=b_t, in_=bg[p])
                # k-sliced, partition-contiguous loads (4 KB per partition
                # line): finer-grain SBUF-write patterns measurably slow
                # concurrent PE operand streams, so keep slices coarse
                if pi == 0:
                    # cold start: sync ring carries only k0 + x, the three
                    # remaining w1 k-slices ride the parallel ACT ring, so
                    # the k-outer first m-group never waits on one ring
                    nc.sync.dma_start(
                        out=w1_t[:, 0, 0:512], in_=w1_s[:, 0, 0:512])
                    nc.sync.dma_start(out=x_t[:, 0, :W], in_=xt_s[:, 0, :W])
                    nc.sync.dma_start(
                        out=w1_t[:, 0, 512:], in_=w1_s[:, 0, 512:])
                    for k in range(1, KTILES1):
                        nc.sync.dma_start(
                            out=x_t[:, k, :W], in_=xt_s[:, k, :W])
                    for k in range(1, KTILES1):
                        nc.scalar.dma_start(out=w1_t[:, k], in_=w1_s[:, k])
                else:
                    for k in range(KTILES1):
                        nc.sync.dma_start(out=w1_t[:, k], in_=w1_s[:, k])
                        nc.sync.dma_start(
                            out=x_t[:, k, :W], in_=xt_s[:, k, :W])
                # w2 also on the ACT ring: needed only once GEMM2 starts
                w2_t = w2p.tile([128, KTILES2, D], BF16, tag="w2")
                for q in range(0, KTILES2, 8):
                    nc.scalar.dma_start(
                        out=w2_t[:, q:q + 8], in_=w2_s[:, q:q + 8])

                h_t = hp.tile([128, KTILES2, CHUNK], BF16, tag="h")
                if pi == 0:
                    # cold start: k-outer / m-group-inner for the first
                    # m-group so weight k-slices are consumed at DMA
                    # supply pace instead of all-at-once
                    pss = [ps_h.tile([128, CHUNK], F32, tag="psh",
                                     name=f"psh{mi}")
                           for mi in range(4)]
                    for k in range(KTILES1):
                        for mi in range(4):
                            nc.tensor.matmul(
                                pss[mi][:, :W],
                                lhsT=w1_t[:, k, mi * 128:(mi + 1) * 128],
                                rhs=x_t[:, k, :W],
                                start=(k == 0),
                                stop=(k == KTILES1 - 1),
                            )
                    for mi in range(4):
                        nc.scalar.activation(
                            h_t[:, mi, :W], pss[mi][:, :W], Gelu,
                            bias=b_t[:, mi:mi + 1]
                        )
                    m_rest = range(4, MTILES1)
                else:
                    m_rest = range(MTILES1)
                for m in m_rest:
                    ps = ps_h.tile([128, CHUNK], F32, tag="psh")
                    for k in range(KTILES1):
                        nc.tensor.matmul(
                            ps[:, :W],
                            lhsT=w1_t[:, k, m * 128:(m + 1) * 128],
                            rhs=x_t[:, k, :W],
                            start=(k == 0),
                            stop=(k == KTILES1 - 1),
                        )
                    nc.scalar.activation(
                        h_t[:, m, :W], ps[:, :W], Gelu, bias=b_t[:, m:m + 1]
                    )

                y_t = yp.tile([128, MTILES2, CHUNK], BF16, tag="y")
                for dm in range(MTILES2):
                    ps2 = ps_y.tile([128, CHUNK], F32, tag="psy")
                    for k in range(KTILES2):
                        nc.tensor.matmul(
                            ps2[:, :W],
                            lhsT=w2_t[:, k, dm * 128:(dm + 1) * 128],
                            rhs=h_t[:, k, :W],
                            start=(k == 0),
                            stop=(k == KTILES2 - 1),
                        )
                    nc.scalar.activation(
                        y_t[:, dm, :W], ps2[:, :W], Ident,
                        bias=b_t[:, MTILES1 + dm:MTILES1 + dm + 1]
                    )
                if pi == len(order) - 1:
                    # final store on the fast HWDGE ring: shorter end drain
                    nc.sync.dma_start(out=yt_s[:, :, :W], in_=y_t[:, :, :W])
                else:
                    nc.gpsimd.dma_start(out=yt_s[:, :, :W], in_=y_t[:, :, :W])

    nc.compile()
    return nc


def kernel(packed_inputs, valid_load, w1, b1, w2, b2, _trace=False, **_):
    global LAST_RESULT
    packed_inputs = np.ascontiguousarray(np.asarray(packed_inputs, np.float32))
    w1 = np.asarray(w1, np.float32)
    b1 = np.asarray(b1, np.float32)
    w2 = np.asarray(w2, np.float32)
    b2 = np.asarray(b2, np.float32)
    v = np.asarray(valid_load).astype(np.int64)
    v = np.clip(v, 0, CAP)

    out = np.zeros((E, CAP, D), np.float32)
    if int(v.max()) <= 0:
        return out

    widths, order, groups = _schedule(v)
    P = len(widths)

    key = (widths, order)
    if key not in _PROGRAM_CACHE:
        _PROGRAM_CACHE[key] = _build_program(widths, order)
    nc = _PROGRAM_CACHE[key]

    w1_bf = w1.astype(NP_BF16)
    w2_bf = w2.astype(NP_BF16)
    xT = packed_inputs.transpose(0, 2, 1)  # [E, D, CAP] view

    in_maps = []
    for c in range(N_CORES):
        xt_h = np.zeros((P, D, CHUNK), NP_BF16)
        w1_h = np.zeros((P, D, H), NP_BF16)
        w2_h = np.zeros((P, H, D), NP_BF16)
        b_h = np.zeros((P, 128, MTILES1 + MTILES2), np.float32)
        for g in range(P):
            if c >= len(groups[g]):
                continue
            e, c0, w = groups[g][c]
            if w <= 0:
                continue
            xt_h[g, :, :w] = xT[e, :, c0:c0 + w].astype(NP_BF16)
            w1_h[g] = w1_bf[e]
            w2_h[g] = w2_bf[e]
            b_h[g, :, :MTILES1] = b1[e].reshape(MTILES1, 128).T
            b_h[g, :, MTILES1:] = b2[e].reshape(MTILES2, 128).T
        in_maps.append({"xt": xt_h, "w1g": w1_h, "w2g": w2_h, "bg": b_h})

    res = run_bass_kernel_spmd(nc, in_maps, list(range(N_CORES)), trace=_trace)
    LAST_RESULT = res

    for c in range(N_CORES):
        ytc = res.results[c]["yt"]
        for g in range(P):
            if c >= len(groups[g]):
                continue
            e, c0, w = groups[g][c]
            weff = min(w, int(v[e]) - c0)
            if weff > 0:
                out[e, c0:c0 + weff, :] = (
                    ytc[g, :, :weff].astype(np.float32).T)
    return out
